# revision 26
# baseline (speedup 1.0000x reference)
"""Trainium2 Bass kernel for a Neural-CA step (depthwise sobel perceive ->
1x1-conv MLP (48->64->64->16) -> masked update -> alive masking), 2 steps,
batch-sharded across 8 NeuronCores (1 image of [16,256,256] per core).

Layout (per core): x lives in SBUF as [128, 32, 258] fp32 where partition
p = g*16 + c (g = 32-row group, c = channel) and free = (row-in-group,
1 + col) with circularly-padded columns at 0 and 257.

Sobel is computed separably on the vector engine in bf16 (vertical taps via
free-dim shifts + DMA-staged group-boundary halos; horizontal taps via
free-dim shifts). The MLP runs on the tensor engine as bf16 matmuls with
block-diagonal weights packing 2 row-groups per matmul (M=128); layer-1 is
3 PSUM-accumulated matmuls (x*W1id + sx*W1sx + sy*W1sy). The scalar engine
fuses relu+bias into the PSUM->SBUF copies. Alive masks are computed in a
compact 2-rows-per-partition layout and replicated to the x layout via a
DRAM-bounce broadcast DMA.
"""

import numpy as np

import concourse.bacc as bacc
import concourse.mybir as mybir
import concourse.tile as tile
from concourse.bass_utils import run_bass_kernel_spmd

f32 = mybir.dt.float32
bf16 = mybir.dt.bfloat16
AOT = mybir.AluOpType
AF = mybir.ActivationFunctionType

N_CORES = 8
C = 16  # channels
H = W = 256
G = 8  # row groups
RG = H // G  # rows per group (32)
WP = W + 2  # padded width
THRESH = 0.1
UPDATE_RATE = 0.25


def _load_x(nc, xt, x_dram):
    for g in range(G):
        nc.sync.dma_start(
            xt[g * C : (g + 1) * C, :, 1:257], x_dram[:, g * RG : (g + 1) * RG, :]
        )
    nc.gpsimd.tensor_copy(xt[:, :, 0:1], xt[:, :, 256:257])
    nc.gpsimd.tensor_copy(xt[:, :, 257:258], xt[:, :, 1:2])


def _alive_compact(nc, pool, ac, pre, tag_prefix):
    """3x3 circular max-pool of compact alpha ac [128,2,258] (f32, cols 1:257
    valid, pads maintained here), threshold > 0.1 -> pre [128,2,256] bf16."""
    nc.gpsimd.tensor_copy(ac[:, :, 0:1], ac[:, :, 256:257])
    nc.gpsimd.tensor_copy(ac[:, :, 257:258], ac[:, :, 1:2])
    hm = pool.tile([128, 2, 256], f32, name=f"{tag_prefix}_hm", tag="mp_hm")
    hm2 = pool.tile([128, 2, 256], f32, name=f"{tag_prefix}_hm2", tag="mp_hm2")
    nc.vector.tensor_tensor(hm[:], ac[:, :, 0:256], ac[:, :, 1:257], op=AOT.max)
    nc.vector.tensor_tensor(hm2[:], hm[:], ac[:, :, 2:258], op=AOT.max)
    tmp = pool.tile([128, 256], f32, name=f"{tag_prefix}_tmp", tag="mp_tmp")
    nc.vector.tensor_tensor(tmp[:], hm2[:, 0, :], hm2[:, 1, :], op=AOT.max)
    shu = pool.tile([128, 256], f32, name=f"{tag_prefix}_shu", tag="mp_shu")
    shd = pool.tile([128, 256], f32, name=f"{tag_prefix}_shd", tag="mp_shd")
    # shu[p] = hm2[p-1, 1], shd[p] = hm2[p+1, 0] (circular partitions)
    nc.sync.dma_start(shu[1:128], hm2[0:127, 1, :])
    nc.sync.dma_start(shu[0:1], hm2[127:128, 1, :])
    nc.sync.dma_start(shd[0:127], hm2[1:128, 0, :])
    nc.sync.dma_start(shd[127:128], hm2[0:1, 0, :])
    m0 = pool.tile([128, 256], f32, name=f"{tag_prefix}_m0", tag="mp_m0")
    m1 = pool.tile([128, 256], f32, name=f"{tag_prefix}_m1", tag="mp_m1")
    nc.vector.tensor_tensor(m0[:], tmp[:], shu[:], op=AOT.max)
    nc.vector.tensor_tensor(m1[:], tmp[:], shd[:], op=AOT.max)
    nc.vector.tensor_scalar(pre[:, 0, :], m0[:], THRESH, None, op0=AOT.is_gt)
    nc.vector.tensor_scalar(pre[:, 1, :], m1[:], THRESH, None, op0=AOT.is_gt)


def build(steps=2):
    nc = bacc.Bacc(None, target_bir_lowering=False)
    x_dram = nc.dram_tensor("x", [C, H, W], f32, kind="ExternalInput")
    w1_d = nc.dram_tensor("w1", [64, 48], f32, kind="ExternalInput")
    b1_d = nc.dram_tensor("b1", [64], f32, kind="ExternalInput")
    w2_d = nc.dram_tensor("w2", [64, 64], f32, kind="ExternalInput")
    b2_d = nc.dram_tensor("b2", [64], f32, kind="ExternalInput")
    w3_d = nc.dram_tensor("w3", [16, 64], f32, kind="ExternalInput")
    um_d = nc.dram_tensor("um", [steps, H, W], f32, kind="ExternalInput")
    out_d = nc.dram_tensor("out", [C, H, W], f32, kind="ExternalOutput")

    with tile.TileContext(nc) as tc:
        with (
            tc.tile_pool(name="pool", bufs=1) as pool,
            tc.tile_pool(name="hpool", bufs=3) as hpool,
            tc.tile_pool(name="cpool", bufs=1) as cpool,
            tc.tile_pool(name="spool", bufs=2) as spool,
            tc.tile_pool(name="ps1", bufs=4, space="PSUM") as ps1,
            tc.tile_pool(name="ps2", bufs=2, space="PSUM") as ps2,
            tc.tile_pool(name="ps3", bufs=2, space="PSUM") as ps3,
            tc.tile_pool(name="dram", bufs=1, space="DRAM") as dpool,
        ):
            # ---------------- weights ----------------
            # perceive channel order per group: 3c+0=ident, 3c+1=sx, 3c+2=sy
            # L1 weights replicated at partition bases 0/32/64/96 so each
            # group-pair matmul has lhsT.base == rhs.base (row-tiled PE).
            wstage = pool.tile([128, 128], f32, name="wstage", tag="wstage")
            w1xT = pool.tile([128, 128], bf16, name="w1xT")
            w1sxT = pool.tile([128, 128], bf16, name="w1sxT")
            w1syT = pool.tile([128, 128], bf16, name="w1syT")
            w2T = pool.tile([128, 128], bf16, name="w2T")
            w3T = pool.tile([128, 32], bf16, name="w3T")
            for k, wt in ((0, w1xT), (1, w1sxT), (2, w1syT)):
                src = w1_d[:].rearrange("o (i k) -> k i o", k=3)[k]  # [16, 64]
                nc.vector.memset(wstage[:], 0.0)
                for q in range(4):
                    nc.sync.dma_start(wstage[q * 32 : q * 32 + 16, 0:64], src)
                    nc.sync.dma_start(wstage[q * 32 + 16 : q * 32 + 32, 64:128], src)
                nc.vector.tensor_copy(wt[:], wstage[:])
            nc.vector.memset(wstage[:], 0.0)
            w2src = w2_d[:].rearrange("o i -> i o")  # [64, 64]
            nc.sync.dma_start(wstage[0:64, 0:64], w2src)
            nc.sync.dma_start(wstage[64:128, 64:128], w2src)
            nc.vector.tensor_copy(w2T[:], wstage[:])
            nc.vector.memset(wstage[:, 0:32], 0.0)
            w3src = w3_d[:].rearrange("o i -> i o")  # [64, 16]
            nc.sync.dma_start(wstage[0:64, 0:16], w3src)
            nc.sync.dma_start(wstage[64:128, 16:32], w3src)
            nc.vector.tensor_copy(w3T[:], wstage[:, 0:32])
            b1t = pool.tile([128, 1], f32, name="b1t")
            b2t = pool.tile([128, 1], f32, name="b2t")
            nc.sync.dma_start(b1t[0:64, :], b1_d[:].unsqueeze(1))
            nc.sync.dma_start(b1t[64:128, :], b1_d[:].unsqueeze(1))
            nc.sync.dma_start(b2t[0:64, :], b2_d[:].unsqueeze(1))
            nc.sync.dma_start(b2t[64:128, :], b2_d[:].unsqueeze(1))

            # ---------------- state ----------------
            xt = pool.tile([128, RG, WP], f32, name="xt")
            _load_x(nc, xt, x_dram)

            for s in range(steps):
                pfx = f"s{s}"
                # ---- bf16 cast of x (incl pads)
                xb = pool.tile([128, RG, WP], bf16, name=f"{pfx}_xb", tag="xb")
                nc.scalar.copy(xb[:], xt[:])

                # ---- group-boundary halos (circular): hu[p]=xb[p-16,31,:],
                # hd[p]=xb[p+16,0,:]
                hu = pool.tile([128, WP], bf16, name=f"{pfx}_hu", tag="hu")
                hd = pool.tile([128, WP], bf16, name=f"{pfx}_hd", tag="hd")
                nc.sync.dma_start(hu[16:128], xb[0:112, RG - 1, :])
                nc.sync.dma_start(hu[0:16], xb[112:128, RG - 1, :])
                nc.sync.dma_start(hd[0:112], xb[16:128, 0, :])
                nc.sync.dma_start(hd[112:128], xb[0:16, 0, :])

                # sobel emitted per row-chunk with chunk-sized transient
                # tiles (bufs=2) so DVE overlaps the PE matmuls
                def sobel_chunk(r0, r1):
                    n = r1 - r0
                    Ac = spool.tile([128, n, WP], bf16, name=f"A_{s}_{r0}", tag="Ac")
                    t2c = spool.tile([128, n, WP], bf16, name=f"t2_{s}_{r0}", tag="t2c")
                    # vertical: A = up+down, t2 = down-up
                    i0, i1 = max(r0, 1), min(r1, RG - 1)
                    nc.vector.tensor_add(Ac[:, i0 - r0 : i1 - r0, :], xb[:, i0 - 1 : i1 - 1, :], xb[:, i0 + 1 : i1 + 1, :])
                    nc.vector.tensor_sub(t2c[:, i0 - r0 : i1 - r0, :], xb[:, i0 + 1 : i1 + 1, :], xb[:, i0 - 1 : i1 - 1, :])
                    if r0 == 0:
                        nc.vector.tensor_add(Ac[:, 0:1, :], hu[:].unsqueeze(1), xb[:, 1:2, :])
                        nc.vector.tensor_sub(t2c[:, 0:1, :], xb[:, 1:2, :], hu[:].unsqueeze(1))
                    if r1 == RG:
                        nc.vector.tensor_add(Ac[:, n - 1 : n, :], xb[:, RG - 2 : RG - 1, :], hd[:].unsqueeze(1))
                        nc.vector.tensor_sub(t2c[:, n - 1 : n, :], hd[:].unsqueeze(1), xb[:, RG - 2 : RG - 1, :])
                    t1c = spool.tile([128, n, WP], bf16, name=f"t1_{s}_{r0}", tag="t1c")
                    nc.vector.scalar_tensor_tensor(
                        t1c[:], xb[:, r0:r1, :], 2.0, Ac[:], op0=AOT.mult, op1=AOT.add
                    )
                    # horizontal (shifted-by-1 storage: col j = image col j)
                    sxc = spool.tile([128, n, W], bf16, name=f"sx_{s}_{r0}", tag="sxc")
                    nc.vector.tensor_sub(sxc[:], t1c[:, :, 2:258], t1c[:, :, 0:256])
                    sy0c = spool.tile([128, n, W], bf16, name=f"sy0_{s}_{r0}", tag="sy0c")
                    nc.vector.tensor_add(sy0c[:], t2c[:, :, 0:256], t2c[:, :, 2:258])
                    syc = spool.tile([128, n, W], bf16, name=f"sy_{s}_{r0}", tag="syc")
                    nc.vector.scalar_tensor_tensor(
                        syc[:], t2c[:, :, 1:257], 2.0, sy0c[:], op0=AOT.mult, op1=AOT.add
                    )
                    return sxc, syc

                # ---- pre-alive mask from current x (compact alpha layout)
                ac = cpool.tile([128, 2, WP], f32, name=f"{pfx}_ac", tag="ac", bufs=1)
                al_d = dpool.tile([H, W], f32, name=f"{pfx}_al_d", tag="al_d")
                alpha_view = xt[:].rearrange("(g c) r w -> g c r w", c=C)[:, 3, :, 1:257]
                nc.sync.dma_start(
                    al_d[:].rearrange("(g r) w -> g r w", g=G), alpha_view
                )
                nc.sync.dma_start(
                    ac[:, :, 1:257], al_d[:].rearrange("(p r) w -> p r w", r=2)
                )
                pre = cpool.tile([128, 2, W], bf16, name=f"{pfx}_pre", tag="pre", bufs=1)
                _alive_compact(nc, cpool, ac, pre, f"{pfx}pre")

                # ---- update-rate mask (compact)
                umc = cpool.tile([128, 2, W], f32, name=f"{pfx}_umc", tag="umc", bufs=1)
                nc.sync.dma_start(umc[:], um_d[s].rearrange("(p r) w -> p r w", r=2))
                umq = cpool.tile([128, 2, W], bf16, name=f"{pfx}_umq", tag="umq", bufs=1)
                nc.vector.tensor_scalar(umq[:], umc[:], UPDATE_RATE, None, op0=AOT.is_lt)

                # ---- MLP over 2-row slices, 2 groups per matmul (M=128)
                dyf = pool.tile([128, RG, W], bf16, name=f"{pfx}_dyf", tag="dyf")
                # L1 emitted weight-major across the 4 row-tiled group-pairs
                # (consecutive matmuls hit different PE row groups, letting
                # the reorder window pull LDWEIGHTS ahead); L2/L3 for slice
                # k-1 are deferred to interleave with slice k's L1.
                def emit_l1(r2):
                    r = 2 * r2
                    rl = r % 8
                    p1s = [
                        ps1.tile([128, 2, 256], f32, name=f"p1_{s}_{r2}_{gp}", tag="l1")
                        for gp in range(4)
                    ]
                    for gp in range(4):
                        sl = slice(gp * 32, (gp + 1) * 32)
                        tp = (gp * 32, 0)
                        nc.tensor.matmul(p1s[gp][:], w1xT[sl], xb[sl, r : r + 2, 1:257], start=True, stop=False, tile_position=tp)
                        nc.tensor.matmul(p1s[gp][:], w1sxT[sl], sxc[sl, rl : rl + 2, :], start=False, stop=False, tile_position=tp)
                        nc.tensor.matmul(p1s[gp][:], w1syT[sl], syc[sl, rl : rl + 2, :], start=False, stop=True, tile_position=tp)
                    return p1s

                def emit_l23(r2, p1s):
                    r = 2 * r2
                    p3 = ps3.tile([128, 2, 256], f32, name=f"p3_{s}_{r2}", tag="l3")
                    for gp in range(4):
                        h1 = hpool.tile([128, 2, 256], bf16, name=f"h1_{s}_{r2}_{gp}", tag="h1")
                        nc.scalar.activation(h1[:], p1s[gp][:], AF.Relu, bias=b1t[:])
                        p2 = ps2.tile([128, 2, 256], f32, name=f"p2_{s}_{r2}_{gp}", tag="l2")
                        nc.tensor.matmul(p2[:], w2T[:], h1[:], start=True, stop=True)
                        h2 = hpool.tile([128, 2, 256], bf16, name=f"h2_{s}_{r2}_{gp}", tag="h2")
                        nc.scalar.activation(h2[:], p2[:], AF.Relu, bias=b2t[:])
                        nc.tensor.matmul(
                            p3[gp * 32 : (gp + 1) * 32], w3T[:], h2[:],
                            start=True, stop=True, tile_position=(0, gp * 32),
                        )
                    nc.scalar.copy(dyf[:, r : r + 2, :], p3[:])

                sxc = syc = None
                for r2 in range(RG // 2):
                    if (2 * r2) % 8 == 0:
                        sxc, syc = sobel_chunk(2 * r2, 2 * r2 + 8)
                    emit_l23(r2, emit_l1(r2))

                # ---- alpha after unmasked update (compact):
                # alpha_v = alpha + dy[ch3]*umq
                dyA = cpool.tile([128, 2, W], bf16, name=f"{pfx}_dyA", tag="dyA", bufs=1)
                dyA_d = dpool.tile([H, W], bf16, name=f"{pfx}_dyA_d", tag="dyA_d")
                dyA_view = dyf[:].rearrange("(g c) r w -> g c r w", c=C)[:, 3]
                nc.sync.dma_start(
                    dyA_d[:].rearrange("(g r) w -> g r w", g=G), dyA_view
                )
                nc.sync.dma_start(dyA[:], dyA_d[:].rearrange("(p r) w -> p r w", r=2))
                dau = cpool.tile([128, 2, W], f32, name=f"{pfx}_dau", tag="dau", bufs=1)
                nc.vector.tensor_mul(dau[:], dyA[:], umq[:])
                av = cpool.tile([128, 2, WP], f32, name=f"{pfx}_av", tag="av", bufs=1)
                nc.vector.tensor_add(av[:, :, 1:257], ac[:, :, 1:257], dau[:])
                post = cpool.tile([128, 2, W], bf16, name=f"{pfx}_post", tag="post", bufs=1)
                _alive_compact(nc, cpool, av, post, f"{pfx}post")

                # ---- combined masks: a = pre*post, ua = umq*a
                am = cpool.tile([128, 2, W], bf16, name=f"{pfx}_am", tag="am", bufs=1)
                nc.vector.tensor_mul(am[:], pre[:], post[:])
                uam = cpool.tile([128, 2, W], bf16, name=f"{pfx}_uam", tag="uam", bufs=1)
                nc.vector.tensor_mul(uam[:], umq[:], am[:])

                # ---- x = x*a + dy*ua  (per 8-row chunk of every group);
                # masks replicated across the 16 channel-partitions of each
                # group via DVE stream_shuffle (blockwise partition gather:
                # within each 32-partition block, compact row-pair rp lives
                # at in-block partitions rp / 16+rp for the two groups).
                CH = 8
                for cc in range(RG // CH):
                    rr = cc * CH
                    arep = cpool.tile([128, CH, W], bf16, name=f"ar_{s}_{cc}", tag="arep", bufs=2)
                    uarep = cpool.tile([128, CH, W], bf16, name=f"uar_{s}_{cc}", tag="uarep", bufs=2)
                    for j in range(CH // 2):
                        rp = rr // 2 + j
                        mask = [rp] * 16 + [16 + rp] * 16
                        nc.vector.stream_shuffle(
                            arep[:, 2 * j : 2 * j + 2, :], am[:], mask
                        )
                        nc.vector.stream_shuffle(
                            uarep[:, 2 * j : 2 * j + 2, :], uam[:], mask
                        )
                    sA = cpool.tile([128, CH, W], f32, name=f"sA_{s}_{cc}", tag="sA", bufs=1)
                    sB = cpool.tile([128, CH, W], f32, name=f"sB_{s}_{cc}", tag="sB", bufs=1)
                    nc.gpsimd.tensor_mul(sA[:], xt[:, rr : rr + CH, 1:257], arep[:])
                    nc.vector.tensor_mul(sB[:], dyf[:, rr : rr + CH, :], uarep[:])
                    nc.vector.tensor_add(xt[:, rr : rr + CH, 1:257], sA[:], sB[:])

                # ---- refresh circular col pads
                nc.gpsimd.tensor_copy(xt[:, :, 0:1], xt[:, :, 256:257])
                nc.gpsimd.tensor_copy(xt[:, :, 257:258], xt[:, :, 1:2])

            # ---------------- store ----------------
            for g in range(G):
                nc.sync.dma_start(
                    out_d[:, g * RG : (g + 1) * RG, :], xt[g * C : (g + 1) * C, :, 1:257]
                )

    nc.compile()
    return nc


_NC_CACHE = {}


def kernel(**inputs) -> np.ndarray:
    x = np.ascontiguousarray(np.asarray(inputs["x"], dtype=np.float32))
    w1 = np.ascontiguousarray(np.asarray(inputs["w1"], dtype=np.float32))
    b1 = np.ascontiguousarray(np.asarray(inputs["b1"], dtype=np.float32))
    w2 = np.ascontiguousarray(np.asarray(inputs["w2"], dtype=np.float32))
    b2 = np.ascontiguousarray(np.asarray(inputs["b2"], dtype=np.float32))
    w3 = np.ascontiguousarray(np.asarray(inputs["w3"], dtype=np.float32))
    um = np.ascontiguousarray(np.asarray(inputs["update_masks"], dtype=np.float32))
    steps = int(inputs["steps"])
    B = x.shape[0]
    assert B == N_CORES and x.shape == (B, C, H, W)

    if steps not in _NC_CACHE:
        _NC_CACHE[steps] = build(steps)
    nc = _NC_CACHE[steps]

    in_maps = [
        {
            "x": x[b],
            "w1": w1,
            "b1": b1,
            "w2": w2,
            "b2": b2,
            "w3": w3,
            "um": np.ascontiguousarray(um[:, b, 0]),
        }
        for b in range(B)
    ]
    res = run_bass_kernel_spmd(nc, in_maps, core_ids=list(range(N_CORES)))
    return np.stack([res.results[b]["out"] for b in range(B)]).astype(np.float32)


# revision 28
# speedup vs baseline: 1.4127x; 1.4127x over previous
"""Trainium2 Bass kernel for a Neural-CA step (depthwise sobel perceive ->
1x1-conv MLP (48->64->64->16) -> masked update -> alive masking), 2 steps,
batch-sharded across 8 NeuronCores (1 image of [16,256,256] per core).

Layout (per core): x lives in SBUF as [128, 32, 258] fp32 where partition
p = g*16 + c (g = 32-row group, c = channel) and free = (row-in-group,
1 + col) with circularly-padded columns at 0 and 257.

Sobel is computed separably on the vector engine in bf16 (vertical taps via
free-dim shifts + DMA-staged group-boundary halos; horizontal taps via
free-dim shifts). The MLP runs on the tensor engine as bf16 matmuls with
block-diagonal weights packing 2 row-groups per matmul (M=128); layer-1 is
3 PSUM-accumulated matmuls (x*W1id + sx*W1sx + sy*W1sy). The scalar engine
fuses relu+bias into the PSUM->SBUF copies. Alive masks are computed in a
compact 2-rows-per-partition layout and replicated to the x layout via a
DRAM-bounce broadcast DMA.
"""

import numpy as np

import concourse.bacc as bacc
import concourse.mybir as mybir
import concourse.tile as tile
from concourse.bass_utils import run_bass_kernel_spmd

f32 = mybir.dt.float32
bf16 = mybir.dt.bfloat16
AOT = mybir.AluOpType
AF = mybir.ActivationFunctionType

N_CORES = 8
C = 16  # channels
H = W = 256
G = 8  # row groups
RG = H // G  # rows per group (32)
WP = W + 2  # padded width
THRESH = 0.1
UPDATE_RATE = 0.25


def _load_x(nc, xt, x_dram):
    for g in range(G):
        nc.sync.dma_start(
            xt[g * C : (g + 1) * C, :, 1:257], x_dram[:, g * RG : (g + 1) * RG, :]
        )
    nc.gpsimd.tensor_copy(xt[:, :, 0:1], xt[:, :, 256:257])
    nc.gpsimd.tensor_copy(xt[:, :, 257:258], xt[:, :, 1:2])


def _alive_compact(nc, pool, ac, pre, tag_prefix):
    """3x3 circular max-pool of compact alpha ac [128,2,258] (f32, cols 1:257
    valid, pads maintained here), threshold > 0.1 -> pre [128,2,256] bf16."""
    nc.gpsimd.tensor_copy(ac[:, :, 0:1], ac[:, :, 256:257])
    nc.gpsimd.tensor_copy(ac[:, :, 257:258], ac[:, :, 1:2])
    hm = pool.tile([128, 2, 256], f32, name=f"{tag_prefix}_hm", tag="mp_hm")
    hm2 = pool.tile([128, 2, 256], f32, name=f"{tag_prefix}_hm2", tag="mp_hm2")
    nc.vector.tensor_tensor(hm[:], ac[:, :, 0:256], ac[:, :, 1:257], op=AOT.max)
    nc.vector.tensor_tensor(hm2[:], hm[:], ac[:, :, 2:258], op=AOT.max)
    tmp = pool.tile([128, 256], f32, name=f"{tag_prefix}_tmp", tag="mp_tmp")
    nc.vector.tensor_tensor(tmp[:], hm2[:, 0, :], hm2[:, 1, :], op=AOT.max)
    shu = pool.tile([128, 256], f32, name=f"{tag_prefix}_shu", tag="mp_shu")
    shd = pool.tile([128, 256], f32, name=f"{tag_prefix}_shd", tag="mp_shd")
    # shu[p] = hm2[p-1, 1], shd[p] = hm2[p+1, 0] (circular partitions)
    nc.sync.dma_start(shu[1:128], hm2[0:127, 1, :])
    nc.sync.dma_start(shu[0:1], hm2[127:128, 1, :])
    nc.sync.dma_start(shd[0:127], hm2[1:128, 0, :])
    nc.sync.dma_start(shd[127:128], hm2[0:1, 0, :])
    m0 = pool.tile([128, 256], f32, name=f"{tag_prefix}_m0", tag="mp_m0")
    m1 = pool.tile([128, 256], f32, name=f"{tag_prefix}_m1", tag="mp_m1")
    nc.vector.tensor_tensor(m0[:], tmp[:], shu[:], op=AOT.max)
    nc.vector.tensor_tensor(m1[:], tmp[:], shd[:], op=AOT.max)
    nc.vector.tensor_scalar(pre[:, 0, :], m0[:], THRESH, None, op0=AOT.is_gt)
    nc.vector.tensor_scalar(pre[:, 1, :], m1[:], THRESH, None, op0=AOT.is_gt)


def build(steps=2):
    nc = bacc.Bacc(None, target_bir_lowering=False)
    x_dram = nc.dram_tensor("x", [C, H, W], f32, kind="ExternalInput")
    w1_d = nc.dram_tensor("w1", [64, 48], f32, kind="ExternalInput")
    b1_d = nc.dram_tensor("b1", [64], f32, kind="ExternalInput")
    w2_d = nc.dram_tensor("w2", [64, 64], f32, kind="ExternalInput")
    b2_d = nc.dram_tensor("b2", [64], f32, kind="ExternalInput")
    w3_d = nc.dram_tensor("w3", [16, 64], f32, kind="ExternalInput")
    um_d = nc.dram_tensor("um", [steps, H, W], f32, kind="ExternalInput")
    out_d = nc.dram_tensor("out", [C, H, W], f32, kind="ExternalOutput")

    with tile.TileContext(nc) as tc:
        with (
            tc.tile_pool(name="pool", bufs=1) as pool,
            tc.tile_pool(name="hpool", bufs=3) as hpool,
            tc.tile_pool(name="cpool", bufs=1) as cpool,
            tc.tile_pool(name="spool", bufs=2) as spool,
            tc.tile_pool(name="ps1", bufs=4, space="PSUM") as ps1,
            tc.tile_pool(name="ps2", bufs=2, space="PSUM") as ps2,
            tc.tile_pool(name="ps3", bufs=2, space="PSUM") as ps3,
            tc.tile_pool(name="dram", bufs=1, space="DRAM") as dpool,
        ):
            # ---------------- weights ----------------
            # perceive channel order per group: 3c+0=ident, 3c+1=sx, 3c+2=sy
            # L1 weights replicated at partition bases 0/32/64/96 so each
            # group-pair matmul has lhsT.base == rhs.base (row-tiled PE).
            wstage = pool.tile([128, 128], f32, name="wstage", tag="wstage")
            w1xT = pool.tile([128, 128], bf16, name="w1xT")
            w1sxT = pool.tile([128, 128], bf16, name="w1sxT")
            w1syT = pool.tile([128, 128], bf16, name="w1syT")
            w2T = pool.tile([128, 128], bf16, name="w2T")
            w3T = pool.tile([128, 32], bf16, name="w3T")
            for k, wt in ((0, w1xT), (1, w1sxT), (2, w1syT)):
                src = w1_d[:].rearrange("o (i k) -> k i o", k=3)[k]  # [16, 64]
                nc.vector.memset(wstage[:], 0.0)
                for q in range(4):
                    nc.sync.dma_start(wstage[q * 32 : q * 32 + 16, 0:64], src)
                    nc.sync.dma_start(wstage[q * 32 + 16 : q * 32 + 32, 64:128], src)
                nc.vector.tensor_copy(wt[:], wstage[:])
            nc.vector.memset(wstage[:], 0.0)
            w2src = w2_d[:].rearrange("o i -> i o")  # [64, 64]
            nc.sync.dma_start(wstage[0:64, 0:64], w2src)
            nc.sync.dma_start(wstage[64:128, 64:128], w2src)
            nc.vector.tensor_copy(w2T[:], wstage[:])
            nc.vector.memset(wstage[:, 0:32], 0.0)
            w3src = w3_d[:].rearrange("o i -> i o")  # [64, 16]
            nc.sync.dma_start(wstage[0:64, 0:16], w3src)
            nc.sync.dma_start(wstage[64:128, 16:32], w3src)
            nc.vector.tensor_copy(w3T[:], wstage[:, 0:32])
            b1t = pool.tile([128, 1], f32, name="b1t")
            b2t = pool.tile([128, 1], f32, name="b2t")
            nc.sync.dma_start(b1t[0:64, :], b1_d[:].unsqueeze(1))
            nc.sync.dma_start(b1t[64:128, :], b1_d[:].unsqueeze(1))
            nc.sync.dma_start(b2t[0:64, :], b2_d[:].unsqueeze(1))
            nc.sync.dma_start(b2t[64:128, :], b2_d[:].unsqueeze(1))

            # ---------------- state ----------------
            xt = pool.tile([128, RG, WP], f32, name="xt")
            _load_x(nc, xt, x_dram)

            for s in range(steps):
                pfx = f"s{s}"
                # ---- bf16 cast of x (incl pads)
                xb = pool.tile([128, RG, WP], bf16, name=f"{pfx}_xb", tag="xb")
                nc.scalar.copy(xb[:], xt[:])

                # ---- group-boundary halos (circular): hu[p]=xb[p-16,31,:],
                # hd[p]=xb[p+16,0,:]
                hu = pool.tile([128, WP], bf16, name=f"{pfx}_hu", tag="hu")
                hd = pool.tile([128, WP], bf16, name=f"{pfx}_hd", tag="hd")
                nc.sync.dma_start(hu[16:128], xb[0:112, RG - 1, :])
                nc.sync.dma_start(hu[0:16], xb[112:128, RG - 1, :])
                nc.sync.dma_start(hd[0:112], xb[16:128, 0, :])
                nc.sync.dma_start(hd[112:128], xb[0:16, 0, :])

                # sobel emitted per row-chunk with chunk-sized transient
                # tiles (bufs=2) so DVE overlaps the PE matmuls
                def sobel_chunk(r0, r1):
                    n = r1 - r0
                    Ac = spool.tile([128, n, WP], bf16, name=f"A_{s}_{r0}", tag="Ac")
                    t2c = spool.tile([128, n, WP], bf16, name=f"t2_{s}_{r0}", tag="t2c")
                    # vertical: A = up+down, t2 = down-up
                    i0, i1 = max(r0, 1), min(r1, RG - 1)
                    nc.vector.tensor_add(Ac[:, i0 - r0 : i1 - r0, :], xb[:, i0 - 1 : i1 - 1, :], xb[:, i0 + 1 : i1 + 1, :])
                    nc.vector.tensor_sub(t2c[:, i0 - r0 : i1 - r0, :], xb[:, i0 + 1 : i1 + 1, :], xb[:, i0 - 1 : i1 - 1, :])
                    if r0 == 0:
                        nc.vector.tensor_add(Ac[:, 0:1, :], hu[:].unsqueeze(1), xb[:, 1:2, :])
                        nc.vector.tensor_sub(t2c[:, 0:1, :], xb[:, 1:2, :], hu[:].unsqueeze(1))
                    if r1 == RG:
                        nc.vector.tensor_add(Ac[:, n - 1 : n, :], xb[:, RG - 2 : RG - 1, :], hd[:].unsqueeze(1))
                        nc.vector.tensor_sub(t2c[:, n - 1 : n, :], hd[:].unsqueeze(1), xb[:, RG - 2 : RG - 1, :])
                    t1c = spool.tile([128, n, WP], bf16, name=f"t1_{s}_{r0}", tag="t1c")
                    nc.vector.scalar_tensor_tensor(
                        t1c[:], xb[:, r0:r1, :], 2.0, Ac[:], op0=AOT.mult, op1=AOT.add
                    )
                    # horizontal (shifted-by-1 storage: col j = image col j)
                    sxc = spool.tile([128, n, W], bf16, name=f"sx_{s}_{r0}", tag="sxc", bufs=3)
                    nc.vector.tensor_sub(sxc[:], t1c[:, :, 2:258], t1c[:, :, 0:256])
                    sy0c = spool.tile([128, n, W], bf16, name=f"sy0_{s}_{r0}", tag="sy0c")
                    nc.vector.tensor_add(sy0c[:], t2c[:, :, 0:256], t2c[:, :, 2:258])
                    syc = spool.tile([128, n, W], bf16, name=f"sy_{s}_{r0}", tag="syc", bufs=3)
                    nc.vector.scalar_tensor_tensor(
                        syc[:], t2c[:, :, 1:257], 2.0, sy0c[:], op0=AOT.mult, op1=AOT.add
                    )
                    return sxc, syc

                # ---- pre-alive mask from current x (compact alpha layout)
                ac = cpool.tile([128, 2, WP], f32, name=f"{pfx}_ac", tag="ac", bufs=1)
                al_d = dpool.tile([H, W], f32, name=f"{pfx}_al_d", tag="al_d")
                alpha_view = xt[:].rearrange("(g c) r w -> g c r w", c=C)[:, 3, :, 1:257]
                nc.sync.dma_start(
                    al_d[:].rearrange("(g r) w -> g r w", g=G), alpha_view
                )
                nc.sync.dma_start(
                    ac[:, :, 1:257], al_d[:].rearrange("(p r) w -> p r w", r=2)
                )
                pre = cpool.tile([128, 2, W], bf16, name=f"{pfx}_pre", tag="pre", bufs=1)
                _alive_compact(nc, cpool, ac, pre, f"{pfx}pre")

                # ---- update-rate mask (compact)
                umc = cpool.tile([128, 2, W], f32, name=f"{pfx}_umc", tag="umc", bufs=1)
                nc.sync.dma_start(umc[:], um_d[s].rearrange("(p r) w -> p r w", r=2))
                umq = cpool.tile([128, 2, W], bf16, name=f"{pfx}_umq", tag="umq", bufs=1)
                nc.vector.tensor_scalar(umq[:], umc[:], UPDATE_RATE, None, op0=AOT.is_lt)

                # ---- MLP over 2-row slices, 2 groups per matmul (M=128)
                dyf = pool.tile([128, RG, W], bf16, name=f"{pfx}_dyf", tag="dyf")
                # L1 emitted weight-major across the 4 row-tiled group-pairs
                # (consecutive matmuls hit different PE row groups, letting
                # the reorder window pull LDWEIGHTS ahead); L2/L3 for slice
                # k-1 are deferred to interleave with slice k's L1.
                def emit_l1(r2):
                    r = 2 * r2
                    rl = r % 8
                    p1s = [
                        ps1.tile([128, 2, 256], f32, name=f"p1_{s}_{r2}_{gp}", tag="l1")
                        for gp in range(4)
                    ]
                    for gp in range(4):
                        sl = slice(gp * 32, (gp + 1) * 32)
                        tp = (gp * 32, 0)
                        nc.tensor.matmul(p1s[gp][:], w1xT[sl], xb[sl, r : r + 2, 1:257], start=True, stop=False, tile_position=tp)
                        nc.tensor.matmul(p1s[gp][:], w1sxT[sl], sxc[sl, rl : rl + 2, :], start=False, stop=False, tile_position=tp)
                        nc.tensor.matmul(p1s[gp][:], w1syT[sl], syc[sl, rl : rl + 2, :], start=False, stop=True, tile_position=tp)
                    return p1s

                def emit_l23(r2, p1s):
                    r = 2 * r2
                    p3 = ps3.tile([128, 2, 256], f32, name=f"p3_{s}_{r2}", tag="l3")
                    for gp in range(4):
                        h1 = hpool.tile([128, 2, 256], bf16, name=f"h1_{s}_{r2}_{gp}", tag="h1")
                        nc.scalar.activation(h1[:], p1s[gp][:], AF.Relu, bias=b1t[:])
                        p2 = ps2.tile([128, 2, 256], f32, name=f"p2_{s}_{r2}_{gp}", tag="l2")
                        nc.tensor.matmul(p2[:], w2T[:], h1[:], start=True, stop=True)
                        h2 = hpool.tile([128, 2, 256], bf16, name=f"h2_{s}_{r2}_{gp}", tag="h2")
                        nc.scalar.activation(h2[:], p2[:], AF.Relu, bias=b2t[:])
                        nc.tensor.matmul(
                            p3[gp * 32 : (gp + 1) * 32], w3T[:], h2[:],
                            start=True, stop=True, tile_position=(0, gp * 32),
                        )
                    nc.scalar.copy(dyf[:, r : r + 2, :], p3[:])

                sxc = syc = None
                for r2 in range(RG // 2):
                    if (2 * r2) % 8 == 0:
                        sxc, syc = sobel_chunk(2 * r2, 2 * r2 + 8)
                    emit_l23(r2, emit_l1(r2))

                # ---- alpha after unmasked update (compact):
                # alpha_v = alpha + dy[ch3]*umq
                dyA = cpool.tile([128, 2, W], bf16, name=f"{pfx}_dyA", tag="dyA", bufs=1)
                dyA_d = dpool.tile([H, W], bf16, name=f"{pfx}_dyA_d", tag="dyA_d")
                dyA_view = dyf[:].rearrange("(g c) r w -> g c r w", c=C)[:, 3]
                nc.sync.dma_start(
                    dyA_d[:].rearrange("(g r) w -> g r w", g=G), dyA_view
                )
                nc.sync.dma_start(dyA[:], dyA_d[:].rearrange("(p r) w -> p r w", r=2))
                dau = cpool.tile([128, 2, W], f32, name=f"{pfx}_dau", tag="dau", bufs=1)
                nc.vector.tensor_mul(dau[:], dyA[:], umq[:])
                av = cpool.tile([128, 2, WP], f32, name=f"{pfx}_av", tag="av", bufs=1)
                nc.vector.tensor_add(av[:, :, 1:257], ac[:, :, 1:257], dau[:])
                post = cpool.tile([128, 2, W], bf16, name=f"{pfx}_post", tag="post", bufs=1)
                _alive_compact(nc, cpool, av, post, f"{pfx}post")

                # ---- combined masks: a = pre*post, ua = umq*a
                am = cpool.tile([128, 2, W], bf16, name=f"{pfx}_am", tag="am", bufs=1)
                nc.vector.tensor_mul(am[:], pre[:], post[:])
                uam = cpool.tile([128, 2, W], bf16, name=f"{pfx}_uam", tag="uam", bufs=1)
                nc.vector.tensor_mul(uam[:], umq[:], am[:])

                # ---- x = x*a + dy*ua  (per 8-row chunk of every group);
                # masks replicated across the 16 channel-partitions of each
                # group via DVE stream_shuffle (blockwise partition gather:
                # within each 32-partition block, compact row-pair rp lives
                # at in-block partitions rp / 16+rp for the two groups).
                CH = 8
                for cc in range(RG // CH):
                    rr = cc * CH
                    arep = cpool.tile([128, CH, W], bf16, name=f"ar_{s}_{cc}", tag="arep", bufs=2)
                    uarep = cpool.tile([128, CH, W], bf16, name=f"uar_{s}_{cc}", tag="uarep", bufs=2)
                    for j in range(CH // 2):
                        rp = rr // 2 + j
                        mask = [rp] * 16 + [16 + rp] * 16
                        nc.vector.stream_shuffle(
                            arep[:, 2 * j : 2 * j + 2, :], am[:], mask
                        )
                        nc.vector.stream_shuffle(
                            uarep[:, 2 * j : 2 * j + 2, :], uam[:], mask
                        )
                    sA = cpool.tile([128, CH, W], f32, name=f"sA_{s}_{cc}", tag="sA", bufs=1)
                    sB = cpool.tile([128, CH, W], f32, name=f"sB_{s}_{cc}", tag="sB", bufs=1)
                    nc.gpsimd.tensor_mul(sA[:], xt[:, rr : rr + CH, 1:257], arep[:])
                    nc.vector.tensor_mul(sB[:], dyf[:, rr : rr + CH, :], uarep[:])
                    nc.vector.tensor_add(xt[:, rr : rr + CH, 1:257], sA[:], sB[:])

                # ---- refresh circular col pads
                nc.gpsimd.tensor_copy(xt[:, :, 0:1], xt[:, :, 256:257])
                nc.gpsimd.tensor_copy(xt[:, :, 257:258], xt[:, :, 1:2])

            # ---------------- store ----------------
            for g in range(G):
                nc.sync.dma_start(
                    out_d[:, g * RG : (g + 1) * RG, :], xt[g * C : (g + 1) * C, :, 1:257]
                )

    nc.compile()
    return nc


_NC_CACHE = {}


def kernel(**inputs) -> np.ndarray:
    x = np.ascontiguousarray(np.asarray(inputs["x"], dtype=np.float32))
    w1 = np.ascontiguousarray(np.asarray(inputs["w1"], dtype=np.float32))
    b1 = np.ascontiguousarray(np.asarray(inputs["b1"], dtype=np.float32))
    w2 = np.ascontiguousarray(np.asarray(inputs["w2"], dtype=np.float32))
    b2 = np.ascontiguousarray(np.asarray(inputs["b2"], dtype=np.float32))
    w3 = np.ascontiguousarray(np.asarray(inputs["w3"], dtype=np.float32))
    um = np.ascontiguousarray(np.asarray(inputs["update_masks"], dtype=np.float32))
    steps = int(inputs["steps"])
    B = x.shape[0]
    assert B == N_CORES and x.shape == (B, C, H, W)

    if steps not in _NC_CACHE:
        _NC_CACHE[steps] = build(steps)
    nc = _NC_CACHE[steps]

    in_maps = [
        {
            "x": x[b],
            "w1": w1,
            "b1": b1,
            "w2": w2,
            "b2": b2,
            "w3": w3,
            "um": np.ascontiguousarray(um[:, b, 0]),
        }
        for b in range(B)
    ]
    res = run_bass_kernel_spmd(nc, in_maps, core_ids=list(range(N_CORES)))
    return np.stack([res.results[b]["out"] for b in range(B)]).astype(np.float32)
